# revision 2
# baseline (speedup 1.0000x reference)
"""Trainium2 Bass kernel for a 4-layer decoder backbone (nn_DecoderBackbone).

Sequence-parallel sharding: core c = (batch b = c//4, rank r = c%4). Each
core owns 256 tokens of its batch -- the two causal chunks r and 7-r (128
tokens each), which balances causal-attention work (every core processes
exactly 12 key-chunk blocks per head: 4 for the low chunk, 8 for the high).

Weights are fully REPLICATED per core in fp16 (norm weights + 1/sqrt(HD)
folded in on the host), streamed from HBM in large per-partition-contiguous
DMAs, so there is no row-parallel reduction anywhere: the only collective is
one AllGather per layer of the local K (rope'd, feature-major) and V (token-
major) over the 4-core batch group, in f32r.

On-device layout: residual x is feature-major [HID, 256] fp32 in SBUF with a
parallel fp16 snapshot that feeds all weight matmuls. Per-token 1/rms is
applied at PSUM eviction (norm commutes with the projections). The attention
path (rope, scores, exp, AV) runs in float32r at full PE rate; projections
run fp16 x fp16. Softmax skips max-subtraction (scores are O(+-10); exp is
safe in fp32). Causal masking is multiplicative post-exp from per-core mask
tables, uniform across cores so one SPMD program serves all 8.
"""
import sys

sys.path.insert(0, "/opt/trn_rl_repo")

import numpy as np

L, B, T, HID = 4, 2, 1024, 2048
NH, NKV, HD = 16, 4, 128
INTER = 5632
EPS = 1e-6
NCORES, GSZ = 8, 4        # 8 cores, 4-core AllGather groups (one per batch)
TOK = 256                 # tokens per core (2 chunks of 128)
KT = HID // 128           # 16 hid tiles
NIT = INTER // 128        # 44 inter tiles
NGQ = 6                   # qkv col groups of 512 (q0..3, k, v)
RG = [[0, 1, 2, 3], [4, 5, 6, 7]]

_CACHE = {}


def _build_program(with_bias, depth_mult=1, fake_coll=False):
    import concourse.bacc as bacc
    import concourse.tile as tile
    import concourse.mybir as mybir
    from contextlib import ExitStack

    F32 = mybir.dt.float32
    F32R = mybir.dt.float32r
    F16 = mybir.dt.float16
    AF = mybir.ActivationFunctionType
    OP = mybir.AluOpType

    nc = bacc.Bacc("TRN2", target_bir_lowering=False, debug=False,
                   num_devices=NCORES)

    XT = nc.dram_tensor("xt_in", [HID, TOK], F32, kind="ExternalInput")
    WQKV = nc.dram_tensor("wqkv", [L, NGQ, 128, KT, 512], F16, kind="ExternalInput")
    WO = nc.dram_tensor("wo", [L, KT, 128, 16, 128], F16, kind="ExternalInput")
    WGU = nc.dram_tensor("wgu", [L, NIT, 128, KT, 256], F16, kind="ExternalInput")
    WD = nc.dram_tensor("wd", [L, KT, 128, NIT, 128], F16, kind="ExternalInput")
    COST = nc.dram_tensor("cost", [HD, TOK], F32, kind="ExternalInput")
    SINST = nc.dram_tensor("sinst", [HD, TOK], F32, kind="ExternalInput")
    MASKA = nc.dram_tensor("maska", [4, 128, 512], F32, kind="ExternalInput")
    MASKB = nc.dram_tensor("maskb", [4, 128, 256], F32, kind="ExternalInput")
    ONES = nc.dram_tensor("ones", [128, 1], F32R, kind="ExternalInput")
    IDT = nc.dram_tensor("idt", [128, 128], F32R, kind="ExternalInput")
    PSW = nc.dram_tensor("psw", [128, 128], F32R, kind="ExternalInput")
    NRMW = nc.dram_tensor("nrmw", [128, KT], F32, kind="ExternalInput")
    EPST = nc.dram_tensor("epst", [1, 1], F32, kind="ExternalInput")
    if with_bias:
        QKVB = nc.dram_tensor("qkvb", [L, 24, 128], F32, kind="ExternalInput")
    OXT = nc.dram_tensor("oxt", [HID, TOK], F32, kind="ExternalOutput")

    with tile.TileContext(nc) as tc, ExitStack() as top:
        persist = top.enter_context(tc.tile_pool(name="persist", bufs=1))
        dram = top.enter_context(tc.tile_pool(name="dram", bufs=2, space="DRAM"))

        xt = persist.tile([128, KT, TOK], F32)
        nc.sync.dma_start(out=xt, in_=XT.ap().rearrange("(k p) t -> p k t", p=128))
        xb = persist.tile([128, KT, TOK], F16)
        cost = persist.tile([128, TOK], F32)
        nc.sync.dma_start(out=cost, in_=COST.ap())
        sinst = persist.tile([128, TOK], F32)
        nc.sync.dma_start(out=sinst, in_=SINST.ap())
        maska = persist.tile([128, 4, 512], F32)
        nc.sync.dma_start(out=maska, in_=MASKA.ap().rearrange("o p f -> p o f"))
        maskb = persist.tile([128, 4, 256], F32)
        nc.sync.dma_start(out=maskb, in_=MASKB.ap().rearrange("o p f -> p o f"))
        ones = persist.tile([128, 1], F32R)
        nc.sync.dma_start(out=ones, in_=ONES.ap())
        idt = persist.tile([128, 128], F32R)
        nc.sync.dma_start(out=idt, in_=IDT.ap())
        psw = persist.tile([128, 128], F32R)
        nc.sync.dma_start(out=psw, in_=PSW.ap())
        nrmw = persist.tile([128, KT], F32)
        nc.sync.dma_start(out=nrmw, in_=NRMW.ap())
        epst = persist.tile([1, 1], F32)
        nc.sync.dma_start(out=epst, in_=EPST.ap())
        if with_bias:
            qkvb = persist.tile([128, L, 24], F32)
            nc.sync.dma_start(out=qkvb, in_=QKVB.ap().rearrange("l c p -> p l c"))

        for k in range(KT):
            nc.scalar.copy(out=xb[:, k, :], in_=xt[:, k, :])

        def norm_scale(pool, pool_ps, sbc):
            # sbc[:] = broadcast rsqrt(mean(x^2) + eps) per token
            var = pool_ps.tile([1, TOK], F32, name="var", bufs=1)
            for k in range(KT):
                sq = pool.tile([128, TOK], F32R, name="sq", bufs=3)
                nc.vector.tensor_tensor(out=sq, in0=xt[:, k, :], in1=xt[:, k, :],
                                        op=OP.mult)
                nc.tensor.matmul(var, ones, sq, start=(k == 0),
                                 stop=(k == KT - 1), skip_group_check=True)
            std = pool.tile([1, TOK], F32, name="std", bufs=1)
            nc.scalar.activation(out=std, in_=var, func=AF.Sqrt,
                                 bias=epst[:, 0:1], scale=1.0 / HID)
            rec = pool.tile([1, TOK], F32, name="rec", bufs=1)
            nc.vector.reciprocal(out=rec, in_=std)
            nc.gpsimd.partition_broadcast(sbc, rec)

        def next_s1():
            # s1 for the NEXT phase-A, from the rotating persist slot
            return persist.tile([128, TOK], F32, name="s1r", bufs=2)

        with ExitStack() as ph0:
            sb0 = ph0.enter_context(tc.tile_pool(name="sb0", bufs=2))
            ps0 = ph0.enter_context(tc.tile_pool(name="ps0", bufs=1, space="PSUM"))
            s1_cur = next_s1()
            norm_scale(sb0, ps0, s1_cur)

        def rope(pool, psR, qs, dst):
            # dst = qs*cos + swap(qs)*sinst  (sign folded into sinst)
            rot = psR.tile([128, TOK], F32, name="rot", bufs=2)
            nc.tensor.matmul(rot, psw, qs, start=True, stop=True,
                             skip_group_check=True)
            qc = pool.tile([128, TOK], F32, name="qc", bufs=3)
            nc.vector.tensor_tensor(out=qc, in0=qs.bitcast(F32), in1=cost,
                                    op=OP.mult)
            rs = pool.tile([128, TOK], F32, name="rs", bufs=3)
            nc.vector.tensor_tensor(out=rs, in0=rot, in1=sinst, op=OP.mult)
            nc.vector.tensor_tensor(out=dst, in0=qc, in1=rs, op=OP.add)

        for l in [li % L for li in range(L * depth_mult)]:
            with ExitStack() as ls:
                sbL = ls.enter_context(tc.tile_pool(name="sbL", bufs=1))
                # kf/vv token order is RANK-major: rank rr's lo chunk at
                # col rr*256, hi chunk at rr*256+128. Global key chunk t:
                #   kcol(t) = t*256 if t < 4 else (7-t)*256 + 128
                #   vtile(t) = 2*t if t < 4 else 2*(7-t) + 1
                qf = sbL.tile([128, NH, TOK], F16, name="qf")
                kf = sbL.tile([128, NKV, 1024], F16, name="kf")
                vv = sbL.tile([128, 8, 512], F32R, name="vv")
                aoT = sbL.tile([128, NH, TOK], F16, name="aoT")

                # ---------- phase A: norm1 + qkv + rope + AllGather ----------
                with ExitStack() as ph:
                    sbA = ph.enter_context(tc.tile_pool(name="sbA", bufs=2))
                    psW = ph.enter_context(tc.tile_pool(name="psW", bufs=4, space="PSUM"))
                    psR = ph.enter_context(tc.tile_pool(name="psR", bufs=2, space="PSUM"))
                    psV = ph.enter_context(tc.tile_pool(name="psV", bufs=2, space="PSUM"))

                    s1 = s1_cur

                    kfL = sbA.tile([128, NKV, TOK], F16, name="kfL", bufs=1)
                    vtL = sbA.tile([128, 2, 512], F16, name="vtL", bufs=1)

                    def qkv_group(g):
                        wq = []
                        for kh in (0, 1):
                            w = sbA.tile([128, 8, 512], F16, name="wq", bufs=3)
                            nc.sync.dma_start(
                                out=w, in_=WQKV.ap()[l, g][:, kh * 8:(kh + 1) * 8, :])
                            wq.append(w)
                        for cc in range(4):
                            ps = psW.tile([128, TOK], F32, name="pqkv", bufs=4)
                            for kt in range(KT):
                                nc.tensor.matmul(
                                    ps, wq[kt // 8][:, kt % 8, cc * 128:(cc + 1) * 128],
                                    xb[:, kt, :], start=(kt == 0), stop=(kt == KT - 1),
                                    skip_group_check=True)
                            qs = sbA.tile([128, TOK], F32R, name="qs", bufs=3)
                            nc.vector.tensor_tensor(out=qs, in0=ps, in1=s1, op=OP.mult)
                            if with_bias:
                                nc.vector.tensor_scalar_add(
                                    out=qs, in0=qs.bitcast(F32),
                                    scalar1=qkvb[:, l, g * 4 + cc:g * 4 + cc + 1])
                            if g < 4:        # q head h = g*4+cc
                                rope(sbA, psR, qs, qf[:, g * 4 + cc, :])
                            elif g == 4:     # k head cc
                                rope(sbA, psR, qs, kfL[:, cc, :])
                            else:            # v head cc: transpose to token-major
                                for j in (0, 1):
                                    pv = psV.tile([128, 128], F32R, name="pv", bufs=2)
                                    nc.tensor.transpose(
                                        pv, qs[:, j * 128:(j + 1) * 128], idt)
                                    nc.scalar.copy(
                                        vtL[:, j, cc * 128:(cc + 1) * 128],
                                        pv.bitcast(F32))

                    # K and V first so the AllGather can fire early
                    qkv_group(4)
                    qkv_group(5)

                    kvin = dram.tile([1024, TOK], F16, name="kvin", bufs=2)
                    nc.sync.dma_start(
                        out=kvin[0:512, :].rearrange("(kv d) t -> d kv t", kv=4),
                        in_=kfL)
                    for tt in (0, 1):
                        nc.sync.dma_start(
                            out=kvin[512 + 256 * tt:512 + 256 * (tt + 1), :]
                            .rearrange("(p hd) m -> p (hd m)", p=128),
                            in_=vtL[:, tt, :])
                    kvout = dram.tile([4096, TOK], F16, name="kvout", bufs=2)
                    if fake_coll:
                        for rr in range(4):
                            nc.sync.dma_start(
                                out=kvout[rr * 1024:(rr + 1) * 1024, :], in_=kvin)
                    else:
                        nc.gpsimd.collective_compute(
                            "AllGather", mybir.AluOpType.bypass, replica_groups=RG,
                            ins=[kvin.opt()], outs=[kvout.opt()])

                    # q heads while the AllGather is in flight
                    for g in range(4):
                        qkv_group(g)

                    # unpack gathered K/V, one DMA each (rank-major token order)
                    vvh = sbA.tile([128, 8, 512], F16, name="vvh", bufs=1)
                    for rr in range(4):
                        nc.sync.dma_start(
                            out=kf.rearrange("d kv (r t) -> d kv r t", r=4)
                            [:, :, rr, :],
                            in_=kvout[rr * 1024:rr * 1024 + 512, :]
                            .rearrange("(kv d) t -> d kv t", kv=4))
                        nc.sync.dma_start(
                            out=vvh[:, 2 * rr:2 * rr + 2, :],
                            in_=kvout[rr * 1024 + 512:(rr + 1) * 1024, :]
                            .rearrange("(tt p hd) m -> p tt (hd m)", tt=2, p=128))
                    for j in range(4):
                        nc.vector.tensor_copy(
                            out=vv[:, 2 * j:2 * j + 2, :],
                            in_=vvh[:, 2 * j:2 * j + 2, :])

                # ---------- phase B: attention + o-proj + residual ----------
                # Head-pairs with 512/256-wide moving operands (f32r needs
                # free>=256 for full PE rate). Column layouts:
                #   sc/ex:  [h0-lo | h0-hi | h1-lo | h1-hi]   (512)
                #   ex2:    [h0-hi | h1-hi]                   (256)
                #   paoL:   [h0-lo | h1-lo]  paoH: [h0-hi | h1-hi]
                with ExitStack() as ph:
                    sbB = ph.enter_context(tc.tile_pool(name="sbB", bufs=2))
                    pha = ph.enter_context(ExitStack())
                    psSc = pha.enter_context(tc.tile_pool(name="psSc", bufs=4, space="PSUM"))
                    psAO = pha.enter_context(tc.tile_pool(name="psAO", bufs=1, space="PSUM"))
                    psSum = pha.enter_context(tc.tile_pool(name="psSum", bufs=1, space="PSUM"))
                    KC = [t * 256 if t < 4 else (7 - t) * 256 + 128
                          for t in range(8)]
                    VT = [2 * t if t < 4 else 2 * (7 - t) + 1 for t in range(8)]
                    for hp in range(NH // 2):
                        kv = hp // 2
                        vkv = vv[:, :, kv * 128:(kv + 1) * 128]
                        paoL = psAO.tile([128, 256], F32, name="paoL", bufs=1)
                        paoH = psAO.tile([128, 256], F32, name="paoH", bufs=1)
                        psmL = psSum.tile([1, 256], F32, name="psmL", bufs=1)
                        psmH = psSum.tile([1, 256], F32, name="psmH", bufs=1)
                        for t in range(4):
                            sc = psSc.tile([128, 512], F32, name="sc", bufs=4)
                            nc.tensor.matmul(
                                sc, kf[:, kv, KC[t]:KC[t] + 128],
                                qf[:, 2 * hp:2 * hp + 2, :],
                                start=True, stop=True, skip_group_check=True)
                            ex = sbB.tile([128, 2, 2, 128], F32R, name="ex", bufs=4)
                            nc.scalar.activation(out=ex, in_=sc, func=AF.Exp)
                            nc.vector.tensor_tensor(
                                out=ex, in0=ex.bitcast(F32),
                                in1=maska[:, t, :], op=OP.mult)
                            exL = ex[:, :, 0, :]   # [128, 2(h), 128] lo cols
                            exH = ex[:, :, 1, :]
                            nc.tensor.matmul(paoL, vkv[:, VT[t]], exL,
                                             start=(t == 0), stop=(t == 3),
                                             skip_group_check=True)
                            nc.tensor.matmul(paoH, vkv[:, VT[t]], exH,
                                             start=(t == 0), stop=False,
                                             skip_group_check=True)
                            nc.tensor.matmul(psmL, ones, exL,
                                             start=(t == 0), stop=(t == 3),
                                             skip_group_check=True)
                            nc.tensor.matmul(psmH, ones, exH,
                                             start=(t == 0), stop=False,
                                             skip_group_check=True)
                        for t in range(4, 8):
                            sc2 = psSc.tile([128, 512], F32, name="sc", bufs=4)
                            nc.tensor.matmul(
                                sc2[:, 0:256], kf[:, kv, KC[t]:KC[t] + 128],
                                qf[:, 2 * hp:2 * hp + 2, 128:256],
                                start=True, stop=True, skip_group_check=True)
                            ex2 = sbB.tile([128, 256], F32R, name="ex2", bufs=4)
                            nc.scalar.activation(out=ex2, in_=sc2[:, 0:256], func=AF.Exp)
                            nc.vector.tensor_tensor(
                                out=ex2, in0=ex2.bitcast(F32),
                                in1=maskb[:, t - 4, :], op=OP.mult)
                            nc.tensor.matmul(paoH, vkv[:, VT[t]], ex2,
                                             start=False, stop=(t == 7),
                                             skip_group_check=True)
                            nc.tensor.matmul(psmH, ones, ex2,
                                             start=False, stop=(t == 7),
                                             skip_group_check=True)
                        rw = sbB.tile([1, 2, 256], F32, name="rw", bufs=2)
                        nc.vector.reciprocal(out=rw[:, 0, :], in_=psmL)
                        nc.vector.reciprocal(out=rw[:, 1, :], in_=psmH)
                        rb = sbB.tile([128, 2, 256], F32, name="rb", bufs=2)
                        nc.gpsimd.partition_broadcast(rb, rw)
                        # aoT[:, 2hp+i, ch*128:...] = pao{L,H}[:, i*128:...] * rb
                        nc.vector.tensor_tensor(
                            out=aoT[:, 2 * hp:2 * hp + 2, 0:128],
                            in0=paoL.rearrange("p (h q) -> p h q", h=2),
                            in1=rb[:, 0, :].rearrange("p (h q) -> p h q", h=2),
                            op=OP.mult)
                        nc.vector.tensor_tensor(
                            out=aoT[:, 2 * hp:2 * hp + 2, 128:256],
                            in0=paoH.rearrange("p (h q) -> p h q", h=2),
                            in1=rb[:, 1, :].rearrange("p (h q) -> p h q", h=2),
                            op=OP.mult)

                    # o-proj (full, local) + residual add
                    pha.close()  # free attention PSUM banks
                    psO = ph.enter_context(tc.tile_pool(name="psO", bufs=2, space="PSUM"))
                    for g in range(KT):
                        wo_t = sbB.tile([128, 16, 128], F16, name="wo_t", bufs=3)
                        nc.sync.dma_start(out=wo_t, in_=WO.ap()[l, g])
                        po = psO.tile([128, TOK], F32, name="po", bufs=2)
                        for ht in range(16):
                            nc.tensor.matmul(po, wo_t[:, ht, :], aoT[:, ht, :],
                                             start=(ht == 0), stop=(ht == 15),
                                             skip_group_check=True)
                        nc.vector.tensor_tensor(out=xt[:, g, :], in0=xt[:, g, :],
                                                in1=po, op=OP.add)
                        nc.scalar.copy(out=xb[:, g, :], in_=xt[:, g, :])

                # ---------- phase C: norm2 + mlp + residual ----------
                with ExitStack() as ph:
                    sbC = ph.enter_context(tc.tile_pool(name="sbC", bufs=2))
                    mTp = ph.enter_context(tc.tile_pool(name="mTp", bufs=1))
                    psG = ph.enter_context(tc.tile_pool(name="psG", bufs=2, space="PSUM"))
                    psU = ph.enter_context(tc.tile_pool(name="psU", bufs=2, space="PSUM"))
                    psD = ph.enter_context(tc.tile_pool(name="psD", bufs=2, space="PSUM"))
                    psS2 = ph.enter_context(tc.tile_pool(name="psS2", bufs=1, space="PSUM"))
                    s2 = sbC.tile([128, TOK], F32, name="s2", bufs=1)
                    norm_scale(sbC, psS2, s2)
                    mT = mTp.tile([128, NIT, TOK], F16, name="mT")
                    for it in range(NIT):
                        wgu_t = sbC.tile([128, KT, 256], F16, name="wgu_t", bufs=3)
                        nc.sync.dma_start(out=wgu_t, in_=WGU.ap()[l, it])
                        pg = psG.tile([128, TOK], F32, name="pg", bufs=2)
                        pu = psU.tile([128, TOK], F32, name="pu", bufs=2)
                        for kt in range(KT):
                            nc.tensor.matmul(pg, wgu_t[:, kt, 0:128], xb[:, kt, :],
                                             start=(kt == 0), stop=(kt == KT - 1),
                                             skip_group_check=True)
                        for kt in range(KT):
                            nc.tensor.matmul(pu, wgu_t[:, kt, 128:256], xb[:, kt, :],
                                             start=(kt == 0), stop=(kt == KT - 1),
                                             skip_group_check=True)
                        gev = sbC.tile([128, TOK], F32, name="gev", bufs=2)
                        nc.vector.tensor_tensor(out=gev, in0=pg, in1=s2, op=OP.mult)
                        gsl = sbC.tile([128, TOK], F32, name="gsl", bufs=2)
                        nc.scalar.activation(out=gsl, in_=gev, func=AF.Silu)
                        uev = sbC.tile([128, TOK], F32, name="uev", bufs=2)
                        nc.vector.tensor_tensor(out=uev, in0=pu, in1=s2, op=OP.mult)
                        nc.vector.tensor_tensor(out=mT[:, it, :], in0=gsl, in1=uev,
                                                op=OP.mult)
                    for g in range(KT):
                        wd_t = sbC.tile([128, NIT, 128], F16, name="wd_t", bufs=3)
                        nc.sync.dma_start(out=wd_t, in_=WD.ap()[l, g])
                        pd = psD.tile([128, TOK], F32, name="pd", bufs=2)
                        for it in range(NIT):
                            nc.tensor.matmul(pd, wd_t[:, it, :], mT[:, it, :],
                                             start=(it == 0), stop=(it == NIT - 1),
                                             skip_group_check=True)
                        nc.vector.tensor_tensor(out=xt[:, g, :], in0=xt[:, g, :],
                                                in1=pd, op=OP.add)
                        nc.scalar.copy(out=xb[:, g, :], in_=xt[:, g, :])
                    # norm scale for the NEXT phase-A (or the final norm),
                    # overlapped with the tail of the down-proj
                    s1_cur = next_s1()
                    norm_scale(sbC, psS2, s1_cur)

        # ---------------- final norm + output ----------------
        with ExitStack() as ph:
            sbF = ph.enter_context(tc.tile_pool(name="sbF", bufs=2))
            sf = s1_cur
            for k in range(KT):
                tmp = sbF.tile([128, TOK], F32, name="tmp", bufs=3)
                nc.vector.tensor_tensor(out=tmp, in0=xt[:, k, :], in1=sf, op=OP.mult)
                ot = sbF.tile([128, TOK], F32, name="ot", bufs=3)
                nc.vector.tensor_scalar_mul(out=ot, in0=tmp, scalar1=nrmw[:, k:k + 1])
                nc.sync.dma_start(out=OXT.ap()[k * 128:(k + 1) * 128, :], in_=ot)

    nc.compile()
    return nc


def _prepare_inputs(inputs):
    g = {k: np.asarray(v) for k, v in inputs.items()}
    qw, kw, vw, ow = g["qw"], g["kw"], g["vw"], g["ow"]
    gatew, upw, downw = g["gatew"], g["upw"], g["downw"]
    ln1w, ln2w, normw = g["ln1w"], g["ln2w"], g["normw"]
    hs, cos, sin = g["hidden_states"], g["cos"], g["sin"]
    qb, kb, vb = g["qb"], g["kb"], g["vb"]

    with_bias = bool(np.any(qb) or np.any(kb) or np.any(vb))
    sc = 1.0 / np.sqrt(HD)

    wqkv = np.empty([L, NGQ, 128, KT, 512], np.float16)
    wo = np.empty([L, KT, 128, 16, 128], np.float16)
    wgu = np.empty([L, NIT, 128, KT, 256], np.float16)
    wd = np.empty([L, KT, 128, NIT, 128], np.float16)
    if with_bias:
        qkvbh = np.empty([L, 24, 128], np.float32)
    for l in range(L):
        cat = np.concatenate([qw[l] * sc, kw[l], vw[l]], axis=0) * ln1w[l][None, :]
        wqkv[l] = cat.reshape(NGQ, 512, KT, 128).transpose(0, 3, 2, 1)
        wo[l] = ow[l].reshape(KT, 128, 16, 128).transpose(0, 3, 2, 1)
        gt = (gatew[l] * ln2w[l][None, :]).reshape(NIT, 128, KT, 128).transpose(0, 3, 2, 1)
        ut = (upw[l] * ln2w[l][None, :]).reshape(NIT, 128, KT, 128).transpose(0, 3, 2, 1)
        wgu[l, :, :, :, 0:128] = gt
        wgu[l, :, :, :, 128:256] = ut
        wd[l] = downw[l].reshape(KT, 128, NIT, 128).transpose(0, 3, 2, 1)
        if with_bias:
            catb = np.concatenate([qb[l] * sc, kb[l], vb[l]], axis=0)
            qkvbh[l] = catb.reshape(24, 128)

    costT = cos[0].T.astype(np.float32)     # [HD, T]
    sinstT = np.concatenate([-sin[0, :, :HD // 2].T, sin[0, :, HD // 2:].T],
                            axis=0).astype(np.float32)
    psw = np.zeros([128, 128], np.float32)
    psw[0:64, 64:128] = np.eye(64)
    psw[64:128, 0:64] = np.eye(64)

    common = {
        "wqkv": wqkv, "wo": wo, "wgu": wgu, "wd": wd,
        "ones": np.ones([128, 1], np.float32),
        "idt": np.eye(128, dtype=np.float32),
        "psw": psw,
        "nrmw": np.ascontiguousarray(normw.reshape(KT, 128).T.astype(np.float32)),
        "epst": np.full([1, 1], EPS, np.float32),
    }
    if with_bias:
        common["qkvb"] = qkvbh

    p = np.arange(128)
    f = np.arange(128)
    in_maps = []
    for c in range(NCORES):
        r, b = c % GSZ, c // GSZ
        cols = np.concatenate([np.arange(128 * r, 128 * (r + 1)),
                               np.arange(128 * (7 - r), 128 * (8 - r))])
        m = dict(common)
        m["xt_in"] = np.ascontiguousarray(hs[b].T[:, cols].astype(np.float32))
        m["cost"] = np.ascontiguousarray(costT[:, cols])
        m["sinst"] = np.ascontiguousarray(sinstT[:, cols])
        # ex is [key-token partitions, query-token cols]: mask[p=k, f=q]
        # maska cols: [h0-lo | h0-hi(ones) | h1-lo | h1-hi(ones)]
        # maskb cols: [h0-hi | h1-hi]
        qlo = 128 * r + f
        qhi = 128 * (7 - r) + f
        maska = np.ones([4, 128, 512], np.float32)
        maskb = np.empty([4, 128, 256], np.float32)
        for t in range(4):
            mlo = (128 * t + p[:, None] <= qlo[None, :]).astype(np.float32)
            maska[t, :, 0:128] = mlo
            maska[t, :, 256:384] = mlo
        for t in range(4, 8):
            mhi = (128 * t + p[:, None] <= qhi[None, :]).astype(np.float32)
            maskb[t - 4, :, 0:128] = mhi
            maskb[t - 4, :, 128:256] = mhi
        m["maska"] = maska
        m["maskb"] = maskb
        in_maps.append(m)
    return in_maps, with_bias


def _get_program(with_bias, depth_mult=1, fake_coll=False):
    key = ("prog", with_bias, depth_mult, fake_coll)
    if key not in _CACHE:
        _CACHE[key] = _build_program(with_bias, depth_mult, fake_coll)
    return _CACHE[key]


def _assemble(res):
    out = np.empty((B, T, HID), np.float32)
    for c in range(NCORES):
        r, b = c % GSZ, c // GSZ
        o = res[c]["oxt"]  # [HID, TOK]
        out[b, 128 * r:128 * (r + 1), :] = o[:, 0:128].T
        out[b, 128 * (7 - r):128 * (8 - r), :] = o[:, 128:256].T
    return out


def kernel(**inputs):
    from concourse import bass_utils
    in_maps, with_bias = _prepare_inputs(inputs)
    nc = _get_program(with_bias)
    r = bass_utils.run_bass_kernel_spmd(nc, in_maps,
                                        core_ids=list(range(NCORES)))
    return np.ascontiguousarray(_assemble(r.results))


# revision 3
# speedup vs baseline: 1.0247x; 1.0247x over previous
"""Trainium2 Bass kernel for a 4-layer decoder backbone (nn_DecoderBackbone).

Sequence-parallel sharding: core c = (batch b = c//4, rank r = c%4). Each
core owns 256 tokens of its batch -- the two causal chunks r and 7-r (128
tokens each), which balances causal-attention work (every core processes
exactly 12 key-chunk blocks per head: 4 for the low chunk, 8 for the high).

Weights are fully REPLICATED per core in fp16 (norm weights + 1/sqrt(HD)
folded in on the host), streamed from HBM in large per-partition-contiguous
DMAs, so there is no row-parallel reduction anywhere: the only collective is
one fp16 AllGather per layer of the local K (rope'd, feature-major) and V
(token-major) over the 4-core batch group.

On-device layout: residual x is feature-major [HID, 256] fp32 in SBUF with a
parallel fp16 snapshot that feeds all weight matmuls. Per-token 1/rms is
applied at PSUM eviction (norm commutes with the projections); each norm
scale is computed at the end of the previous phase so evictions never stall.
Scores run fp16 x fp16; exp/AV run in f32r with moving dims >= 256 (full PE
rate); head-PAIRS share one 512-wide score tile. Softmax skips
max-subtraction (scores are O(+-10); exp is safe in fp32). Causal masking is
multiplicative post-exp from per-core mask tables, uniform across cores so
one SPMD program serves all 8.
"""
import sys

sys.path.insert(0, "/opt/trn_rl_repo")

import numpy as np

L, B, T, HID = 4, 2, 1024, 2048
NH, NKV, HD = 16, 4, 128
INTER = 5632
EPS = 1e-6
NCORES, GSZ = 8, 4        # 8 cores, 4-core AllGather groups (one per batch)
TOK = 256                 # tokens per core (2 chunks of 128)
KT = HID // 128           # 16 hid tiles
NIT = INTER // 128        # 44 inter tiles
NGQ = 6                   # qkv col groups of 512 (q0..3, k, v)
RG = [[0, 1, 2, 3], [4, 5, 6, 7]]

_CACHE = {}


def _build_program(with_bias, depth_mult=1, fake_coll=False):
    import concourse.bacc as bacc
    import concourse.tile as tile
    import concourse.mybir as mybir
    from contextlib import ExitStack

    F32 = mybir.dt.float32
    F32R = mybir.dt.float32r
    F16 = mybir.dt.float16
    AF = mybir.ActivationFunctionType
    OP = mybir.AluOpType

    nc = bacc.Bacc("TRN2", target_bir_lowering=False, debug=False,
                   num_devices=NCORES)

    XT = nc.dram_tensor("xt_in", [HID, TOK], F32, kind="ExternalInput")
    WQKV = nc.dram_tensor("wqkv", [L, NGQ, 128, KT, 512], F16, kind="ExternalInput")
    WO = nc.dram_tensor("wo", [L, KT, 128, 16, 128], F16, kind="ExternalInput")
    WGU = nc.dram_tensor("wgu", [L, NIT, 128, KT, 256], F16, kind="ExternalInput")
    WD = nc.dram_tensor("wd", [L, KT, 128, NIT, 128], F16, kind="ExternalInput")
    COST = nc.dram_tensor("cost", [HD, TOK], F32, kind="ExternalInput")
    SINST = nc.dram_tensor("sinst", [HD, TOK], F32, kind="ExternalInput")
    MASKA = nc.dram_tensor("maska", [4, 128, 512], F32, kind="ExternalInput")
    MASKB = nc.dram_tensor("maskb", [4, 128, 256], F32, kind="ExternalInput")
    ONES = nc.dram_tensor("ones", [128, 1], F32R, kind="ExternalInput")
    IDT = nc.dram_tensor("idt", [128, 128], F32R, kind="ExternalInput")
    PSW = nc.dram_tensor("psw", [128, 128], F32R, kind="ExternalInput")
    NRMW = nc.dram_tensor("nrmw", [128, KT], F32, kind="ExternalInput")
    EPST = nc.dram_tensor("epst", [1, 1], F32, kind="ExternalInput")
    if with_bias:
        QKVB = nc.dram_tensor("qkvb", [L, 24, 128], F32, kind="ExternalInput")
    OXT = nc.dram_tensor("oxt", [HID, TOK], F32, kind="ExternalOutput")

    with tile.TileContext(nc) as tc, ExitStack() as top:
        persist = top.enter_context(tc.tile_pool(name="persist", bufs=1))
        dram = top.enter_context(tc.tile_pool(name="dram", bufs=2, space="DRAM"))

        xt = persist.tile([128, KT, TOK], F32)
        nc.sync.dma_start(out=xt, in_=XT.ap().rearrange("(k p) t -> p k t", p=128))
        xb = persist.tile([128, KT, TOK], F16)
        cost = persist.tile([128, TOK], F32)
        nc.sync.dma_start(out=cost, in_=COST.ap())
        sinst = persist.tile([128, TOK], F32)
        nc.sync.dma_start(out=sinst, in_=SINST.ap())
        maska = persist.tile([128, 4, 512], F32)
        nc.sync.dma_start(out=maska, in_=MASKA.ap().rearrange("o p f -> p o f"))
        maskb = persist.tile([128, 4, 256], F32)
        nc.sync.dma_start(out=maskb, in_=MASKB.ap().rearrange("o p f -> p o f"))
        ones = persist.tile([128, 1], F32R)
        nc.sync.dma_start(out=ones, in_=ONES.ap())
        idt = persist.tile([128, 128], F32R)
        nc.sync.dma_start(out=idt, in_=IDT.ap())
        psw = persist.tile([128, 128], F32R)
        nc.sync.dma_start(out=psw, in_=PSW.ap())
        nrmw = persist.tile([128, KT], F32)
        nc.sync.dma_start(out=nrmw, in_=NRMW.ap())
        epst = persist.tile([1, 1], F32)
        nc.sync.dma_start(out=epst, in_=EPST.ap())
        if with_bias:
            qkvb = persist.tile([128, L, 24], F32)
            nc.sync.dma_start(out=qkvb, in_=QKVB.ap().rearrange("l c p -> p l c"))

        for k in range(KT):
            nc.scalar.copy(out=xb[:, k, :], in_=xt[:, k, :])

        def norm_scale(pool, pool_ps, sbc):
            # sbc[:] = broadcast rsqrt(mean(x^2) + eps) per token
            var = pool_ps.tile([1, TOK], F32, name="var", bufs=1)
            for k in range(KT):
                sq = pool.tile([128, TOK], F32R, name="sq", bufs=3)
                nc.vector.tensor_tensor(out=sq, in0=xt[:, k, :], in1=xt[:, k, :],
                                        op=OP.mult)
                nc.tensor.matmul(var, ones, sq, start=(k == 0),
                                 stop=(k == KT - 1), skip_group_check=True)
            std = pool.tile([1, TOK], F32, name="std", bufs=1)
            nc.scalar.activation(out=std, in_=var, func=AF.Sqrt,
                                 bias=epst[:, 0:1], scale=1.0 / HID)
            rec = pool.tile([1, TOK], F32, name="rec", bufs=1)
            nc.vector.reciprocal(out=rec, in_=std)
            nc.gpsimd.partition_broadcast(sbc, rec)

        def next_s1():
            # s1 for the NEXT phase-A, from the rotating persist slot
            return persist.tile([128, TOK], F32, name="s1r", bufs=2)

        with ExitStack() as ph0:
            sb0 = ph0.enter_context(tc.tile_pool(name="sb0", bufs=2))
            ps0 = ph0.enter_context(tc.tile_pool(name="ps0", bufs=1, space="PSUM"))
            s1_cur = next_s1()
            norm_scale(sb0, ps0, s1_cur)

        def rope(pool, psR, qs, dst):
            # dst = qs*cos + swap(qs)*sinst  (sign folded into sinst)
            rot = psR.tile([128, TOK], F32, name="rot", bufs=2)
            nc.tensor.matmul(rot, psw, qs, start=True, stop=True,
                             skip_group_check=True)
            qc = pool.tile([128, TOK], F32, name="qc", bufs=3)
            nc.vector.tensor_tensor(out=qc, in0=qs.bitcast(F32), in1=cost,
                                    op=OP.mult)
            rs = pool.tile([128, TOK], F32, name="rs", bufs=3)
            nc.vector.tensor_tensor(out=rs, in0=rot, in1=sinst, op=OP.mult)
            nc.vector.tensor_tensor(out=dst, in0=qc, in1=rs, op=OP.add)

        for l in [li % L for li in range(L * depth_mult)]:
            with ExitStack() as ls:
                sbL = ls.enter_context(tc.tile_pool(name="sbL", bufs=1))
                # kf/vv token order is RANK-major: rank rr's lo chunk at
                # col rr*256, hi chunk at rr*256+128. Global key chunk t:
                #   kcol(t) = t*256 if t < 4 else (7-t)*256 + 128
                #   vtile(t) = 2*t if t < 4 else 2*(7-t) + 1
                qf = sbL.tile([128, NH, TOK], F16, name="qf")
                kf = sbL.tile([128, NKV, 1024], F16, name="kf")
                vv = sbL.tile([128, 8, 512], F32R, name="vv")
                aoT = sbL.tile([128, NH, TOK], F16, name="aoT")

                # ---------- phase A: norm1 + qkv + rope + AllGather ----------
                with ExitStack() as ph:
                    sbA = ph.enter_context(tc.tile_pool(name="sbA", bufs=2))
                    psW = ph.enter_context(tc.tile_pool(name="psW", bufs=4, space="PSUM"))
                    psR = ph.enter_context(tc.tile_pool(name="psR", bufs=2, space="PSUM"))
                    psV = ph.enter_context(tc.tile_pool(name="psV", bufs=2, space="PSUM"))

                    s1 = s1_cur

                    kfL = sbA.tile([128, NKV, TOK], F16, name="kfL", bufs=1)
                    vtL = sbA.tile([128, 2, 512], F16, name="vtL", bufs=1)

                    def qkv_group(g):
                        wq = []
                        for kh in (0, 1):
                            w = sbA.tile([128, 8, 512], F16, name="wq", bufs=3)
                            nc.sync.dma_start(
                                out=w, in_=WQKV.ap()[l, g][:, kh * 8:(kh + 1) * 8, :])
                            wq.append(w)
                        for cc in range(4):
                            ps = psW.tile([128, TOK], F32, name="pqkv", bufs=4)
                            for kt in range(KT):
                                nc.tensor.matmul(
                                    ps, wq[kt // 8][:, kt % 8, cc * 128:(cc + 1) * 128],
                                    xb[:, kt, :], start=(kt == 0), stop=(kt == KT - 1),
                                    skip_group_check=True)
                            qs = sbA.tile([128, TOK], F32R, name="qs", bufs=3)
                            nc.vector.tensor_tensor(out=qs, in0=ps, in1=s1, op=OP.mult)
                            if with_bias:
                                nc.vector.tensor_scalar_add(
                                    out=qs, in0=qs.bitcast(F32),
                                    scalar1=qkvb[:, l, g * 4 + cc:g * 4 + cc + 1])
                            if g < 4:        # q head h = g*4+cc
                                rope(sbA, psR, qs, qf[:, g * 4 + cc, :])
                            elif g == 4:     # k head cc
                                rope(sbA, psR, qs, kfL[:, cc, :])
                            else:            # v head cc: transpose to token-major
                                for j in (0, 1):
                                    pv = psV.tile([128, 128], F32R, name="pv", bufs=2)
                                    nc.tensor.transpose(
                                        pv, qs[:, j * 128:(j + 1) * 128], idt)
                                    nc.scalar.copy(
                                        vtL[:, j, cc * 128:(cc + 1) * 128],
                                        pv.bitcast(F32))

                    # K and V first so the AllGather can fire early
                    qkv_group(4)
                    qkv_group(5)

                    kvin = dram.tile([1024, TOK], F16, name="kvin", bufs=2)
                    nc.sync.dma_start(
                        out=kvin[0:512, :].rearrange("(kv d) t -> d kv t", kv=4),
                        in_=kfL)
                    for tt in (0, 1):
                        nc.sync.dma_start(
                            out=kvin[512 + 256 * tt:512 + 256 * (tt + 1), :]
                            .rearrange("(p hd) m -> p (hd m)", p=128),
                            in_=vtL[:, tt, :])
                    kvout = dram.tile([4096, TOK], F16, name="kvout", bufs=2)
                    if fake_coll:
                        for rr in range(4):
                            nc.sync.dma_start(
                                out=kvout[rr * 1024:(rr + 1) * 1024, :], in_=kvin)
                    else:
                        nc.gpsimd.collective_compute(
                            "AllGather", mybir.AluOpType.bypass, replica_groups=RG,
                            ins=[kvin.opt()], outs=[kvout.opt()])

                    # q heads while the AllGather is in flight
                    for g in range(4):
                        qkv_group(g)

                    # unpack gathered K/V, one DMA each (rank-major token order)
                    vvh = sbA.tile([128, 8, 512], F16, name="vvh", bufs=1)
                    for rr in range(4):
                        nc.sync.dma_start(
                            out=kf.rearrange("d kv (r t) -> d kv r t", r=4)
                            [:, :, rr, :],
                            in_=kvout[rr * 1024:rr * 1024 + 512, :]
                            .rearrange("(kv d) t -> d kv t", kv=4))
                        nc.sync.dma_start(
                            out=vvh[:, 2 * rr:2 * rr + 2, :],
                            in_=kvout[rr * 1024 + 512:(rr + 1) * 1024, :]
                            .rearrange("(tt p hd) m -> p tt (hd m)", tt=2, p=128))
                    for j in range(4):
                        nc.vector.tensor_copy(
                            out=vv[:, 2 * j:2 * j + 2, :],
                            in_=vvh[:, 2 * j:2 * j + 2, :])

                # ---------- phase B: attention + o-proj + residual ----------
                # Head-pairs with 512/256-wide moving operands (f32r needs
                # free>=256 for full PE rate). Column layouts:
                #   sc/ex:  [h0-lo | h0-hi | h1-lo | h1-hi]   (512)
                #   ex2:    [h0-hi | h1-hi]                   (256)
                #   paoL:   [h0-lo | h1-lo]  paoH: [h0-hi | h1-hi]
                with ExitStack() as ph:
                    sbB = ph.enter_context(tc.tile_pool(name="sbB", bufs=2))
                    pha = ph.enter_context(ExitStack())
                    psSc = pha.enter_context(tc.tile_pool(name="psSc", bufs=4, space="PSUM"))
                    psAO = pha.enter_context(tc.tile_pool(name="psAO", bufs=1, space="PSUM"))
                    psSum = pha.enter_context(tc.tile_pool(name="psSum", bufs=1, space="PSUM"))
                    KC = [t * 256 if t < 4 else (7 - t) * 256 + 128
                          for t in range(8)]
                    VT = [2 * t if t < 4 else 2 * (7 - t) + 1 for t in range(8)]
                    for hp in range(NH // 2):
                        kv = hp // 2
                        vkv = vv[:, :, kv * 128:(kv + 1) * 128]
                        paoL = psAO.tile([128, 256], F32, name="paoL", bufs=1)
                        paoH = psAO.tile([128, 256], F32, name="paoH", bufs=1)
                        psmL = psSum.tile([1, 256], F32, name="psmL", bufs=1)
                        psmH = psSum.tile([1, 256], F32, name="psmH", bufs=1)
                        for t in range(4):
                            sc = psSc.tile([128, 512], F32, name="sc", bufs=4)
                            nc.tensor.matmul(
                                sc, kf[:, kv, KC[t]:KC[t] + 128],
                                qf[:, 2 * hp:2 * hp + 2, :],
                                start=True, stop=True, skip_group_check=True)
                            ex = sbB.tile([128, 2, 2, 128], F32R, name="ex", bufs=4)
                            nc.scalar.activation(out=ex, in_=sc, func=AF.Exp)
                            nc.vector.tensor_tensor(
                                out=ex, in0=ex.bitcast(F32),
                                in1=maska[:, t, :], op=OP.mult)
                            exL = ex[:, :, 0, :]   # [128, 2(h), 128] lo cols
                            exH = ex[:, :, 1, :]
                            nc.tensor.matmul(paoL, vkv[:, VT[t]], exL,
                                             start=(t == 0), stop=(t == 3),
                                             skip_group_check=True)
                            nc.tensor.matmul(paoH, vkv[:, VT[t]], exH,
                                             start=(t == 0), stop=False,
                                             skip_group_check=True)
                            nc.tensor.matmul(psmL, ones, exL,
                                             start=(t == 0), stop=(t == 3),
                                             skip_group_check=True)
                            nc.tensor.matmul(psmH, ones, exH,
                                             start=(t == 0), stop=False,
                                             skip_group_check=True)
                        for t in range(4, 8):
                            sc2 = psSc.tile([128, 512], F32, name="sc", bufs=4)
                            nc.tensor.matmul(
                                sc2[:, 0:256], kf[:, kv, KC[t]:KC[t] + 128],
                                qf[:, 2 * hp:2 * hp + 2, 128:256],
                                start=True, stop=True, skip_group_check=True)
                            ex2 = sbB.tile([128, 256], F32R, name="ex2", bufs=4)
                            nc.scalar.activation(out=ex2, in_=sc2[:, 0:256], func=AF.Exp)
                            nc.vector.tensor_tensor(
                                out=ex2, in0=ex2.bitcast(F32),
                                in1=maskb[:, t - 4, :], op=OP.mult)
                            nc.tensor.matmul(paoH, vkv[:, VT[t]], ex2,
                                             start=False, stop=(t == 7),
                                             skip_group_check=True)
                            nc.tensor.matmul(psmH, ones, ex2,
                                             start=False, stop=(t == 7),
                                             skip_group_check=True)
                        rw = sbB.tile([1, 2, 256], F32, name="rw", bufs=2)
                        nc.vector.reciprocal(out=rw[:, 0, :], in_=psmL)
                        nc.vector.reciprocal(out=rw[:, 1, :], in_=psmH)
                        rb = sbB.tile([128, 2, 256], F32, name="rb", bufs=2)
                        nc.gpsimd.partition_broadcast(rb, rw)
                        # aoT[:, 2hp+i, ch*128:...] = pao{L,H}[:, i*128:...] * rb
                        nc.vector.tensor_tensor(
                            out=aoT[:, 2 * hp:2 * hp + 2, 0:128],
                            in0=paoL.rearrange("p (h q) -> p h q", h=2),
                            in1=rb[:, 0, :].rearrange("p (h q) -> p h q", h=2),
                            op=OP.mult)
                        nc.vector.tensor_tensor(
                            out=aoT[:, 2 * hp:2 * hp + 2, 128:256],
                            in0=paoH.rearrange("p (h q) -> p h q", h=2),
                            in1=rb[:, 1, :].rearrange("p (h q) -> p h q", h=2),
                            op=OP.mult)

                    # o-proj (full, local) + residual add
                    pha.close()  # free attention PSUM banks
                    psO = ph.enter_context(tc.tile_pool(name="psO", bufs=2, space="PSUM"))
                    for g in range(KT):
                        wo_t = sbB.tile([128, 16, 128], F16, name="wo_t", bufs=3)
                        nc.sync.dma_start(out=wo_t, in_=WO.ap()[l, g])
                        po = psO.tile([128, TOK], F32, name="po", bufs=2)
                        for ht in range(16):
                            nc.tensor.matmul(po, wo_t[:, ht, :], aoT[:, ht, :],
                                             start=(ht == 0), stop=(ht == 15),
                                             skip_group_check=True)
                        nc.vector.tensor_tensor(out=xt[:, g, :], in0=xt[:, g, :],
                                                in1=po, op=OP.add)
                        nc.scalar.copy(out=xb[:, g, :], in_=xt[:, g, :])

                # ---------- phase C: norm2 + mlp + residual ----------
                with ExitStack() as ph:
                    sbC = ph.enter_context(tc.tile_pool(name="sbC", bufs=2))
                    mTp = ph.enter_context(tc.tile_pool(name="mTp", bufs=1))
                    psG = ph.enter_context(tc.tile_pool(name="psG", bufs=2, space="PSUM"))
                    psU = ph.enter_context(tc.tile_pool(name="psU", bufs=2, space="PSUM"))
                    psD = ph.enter_context(tc.tile_pool(name="psD", bufs=2, space="PSUM"))
                    psS2 = ph.enter_context(tc.tile_pool(name="psS2", bufs=1, space="PSUM"))
                    s2 = sbC.tile([128, TOK], F32, name="s2", bufs=1)
                    norm_scale(sbC, psS2, s2)
                    mT = mTp.tile([128, NIT, TOK], F16, name="mT")
                    for it in range(NIT):
                        wgu_t = sbC.tile([128, KT, 256], F16, name="wgu_t", bufs=3)
                        nc.sync.dma_start(out=wgu_t, in_=WGU.ap()[l, it])
                        pg = psG.tile([128, TOK], F32, name="pg", bufs=2)
                        pu = psU.tile([128, TOK], F32, name="pu", bufs=2)
                        for kt in range(KT):
                            nc.tensor.matmul(pg, wgu_t[:, kt, 0:128], xb[:, kt, :],
                                             start=(kt == 0), stop=(kt == KT - 1),
                                             skip_group_check=True)
                        for kt in range(KT):
                            nc.tensor.matmul(pu, wgu_t[:, kt, 128:256], xb[:, kt, :],
                                             start=(kt == 0), stop=(kt == KT - 1),
                                             skip_group_check=True)
                        gev = sbC.tile([128, TOK], F32, name="gev", bufs=2)
                        nc.vector.tensor_tensor(out=gev, in0=pg, in1=s2, op=OP.mult)
                        gsl = sbC.tile([128, TOK], F32, name="gsl", bufs=2)
                        nc.scalar.activation(out=gsl, in_=gev, func=AF.Silu)
                        uev = sbC.tile([128, TOK], F32, name="uev", bufs=2)
                        nc.vector.tensor_tensor(out=uev, in0=pu, in1=s2, op=OP.mult)
                        nc.vector.tensor_tensor(out=mT[:, it, :], in0=gsl, in1=uev,
                                                op=OP.mult)
                    for g in range(KT):
                        wd_t = sbC.tile([128, NIT, 128], F16, name="wd_t", bufs=3)
                        nc.sync.dma_start(out=wd_t, in_=WD.ap()[l, g])
                        pd = psD.tile([128, TOK], F32, name="pd", bufs=2)
                        for it in range(NIT):
                            nc.tensor.matmul(pd, wd_t[:, it, :], mT[:, it, :],
                                             start=(it == 0), stop=(it == NIT - 1),
                                             skip_group_check=True)
                        nc.vector.tensor_tensor(out=xt[:, g, :], in0=xt[:, g, :],
                                                in1=pd, op=OP.add)
                        nc.scalar.copy(out=xb[:, g, :], in_=xt[:, g, :])
                    # norm scale for the NEXT phase-A (or the final norm),
                    # overlapped with the tail of the down-proj
                    s1_cur = next_s1()
                    norm_scale(sbC, psS2, s1_cur)

        # ---------------- final norm + output ----------------
        with ExitStack() as ph:
            sbF = ph.enter_context(tc.tile_pool(name="sbF", bufs=2))
            sf = s1_cur
            for k in range(KT):
                tmp = sbF.tile([128, TOK], F32, name="tmp", bufs=3)
                nc.vector.tensor_tensor(out=tmp, in0=xt[:, k, :], in1=sf, op=OP.mult)
                ot = sbF.tile([128, TOK], F32, name="ot", bufs=3)
                nc.vector.tensor_scalar_mul(out=ot, in0=tmp, scalar1=nrmw[:, k:k + 1])
                nc.sync.dma_start(out=OXT.ap()[k * 128:(k + 1) * 128, :], in_=ot)

    nc.compile()
    return nc


def _prepare_inputs(inputs):
    g = {k: np.asarray(v) for k, v in inputs.items()}
    qw, kw, vw, ow = g["qw"], g["kw"], g["vw"], g["ow"]
    gatew, upw, downw = g["gatew"], g["upw"], g["downw"]
    ln1w, ln2w, normw = g["ln1w"], g["ln2w"], g["normw"]
    hs, cos, sin = g["hidden_states"], g["cos"], g["sin"]
    qb, kb, vb = g["qb"], g["kb"], g["vb"]

    with_bias = bool(np.any(qb) or np.any(kb) or np.any(vb))
    sc = 1.0 / np.sqrt(HD)

    wqkv = np.empty([L, NGQ, 128, KT, 512], np.float16)
    wo = np.empty([L, KT, 128, 16, 128], np.float16)
    wgu = np.empty([L, NIT, 128, KT, 256], np.float16)
    wd = np.empty([L, KT, 128, NIT, 128], np.float16)
    if with_bias:
        qkvbh = np.empty([L, 24, 128], np.float32)
    for l in range(L):
        cat = np.concatenate([qw[l] * sc, kw[l], vw[l]], axis=0) * ln1w[l][None, :]
        wqkv[l] = cat.reshape(NGQ, 512, KT, 128).transpose(0, 3, 2, 1)
        wo[l] = ow[l].reshape(KT, 128, 16, 128).transpose(0, 3, 2, 1)
        gt = (gatew[l] * ln2w[l][None, :]).reshape(NIT, 128, KT, 128).transpose(0, 3, 2, 1)
        ut = (upw[l] * ln2w[l][None, :]).reshape(NIT, 128, KT, 128).transpose(0, 3, 2, 1)
        wgu[l, :, :, :, 0:128] = gt
        wgu[l, :, :, :, 128:256] = ut
        wd[l] = downw[l].reshape(KT, 128, NIT, 128).transpose(0, 3, 2, 1)
        if with_bias:
            catb = np.concatenate([qb[l] * sc, kb[l], vb[l]], axis=0)
            qkvbh[l] = catb.reshape(24, 128)

    costT = cos[0].T.astype(np.float32)     # [HD, T]
    sinstT = np.concatenate([-sin[0, :, :HD // 2].T, sin[0, :, HD // 2:].T],
                            axis=0).astype(np.float32)
    psw = np.zeros([128, 128], np.float32)
    psw[0:64, 64:128] = np.eye(64)
    psw[64:128, 0:64] = np.eye(64)

    common = {
        "wqkv": wqkv, "wo": wo, "wgu": wgu, "wd": wd,
        "ones": np.ones([128, 1], np.float32),
        "idt": np.eye(128, dtype=np.float32),
        "psw": psw,
        "nrmw": np.ascontiguousarray(normw.reshape(KT, 128).T.astype(np.float32)),
        "epst": np.full([1, 1], EPS, np.float32),
    }
    if with_bias:
        common["qkvb"] = qkvbh

    p = np.arange(128)
    f = np.arange(128)
    in_maps = []
    for c in range(NCORES):
        r, b = c % GSZ, c // GSZ
        cols = np.concatenate([np.arange(128 * r, 128 * (r + 1)),
                               np.arange(128 * (7 - r), 128 * (8 - r))])
        m = dict(common)
        m["xt_in"] = np.ascontiguousarray(hs[b].T[:, cols].astype(np.float32))
        m["cost"] = np.ascontiguousarray(costT[:, cols])
        m["sinst"] = np.ascontiguousarray(sinstT[:, cols])
        # ex is [key-token partitions, query-token cols]: mask[p=k, f=q]
        # maska cols: [h0-lo | h0-hi(ones) | h1-lo | h1-hi(ones)]
        # maskb cols: [h0-hi | h1-hi]
        qlo = 128 * r + f
        qhi = 128 * (7 - r) + f
        maska = np.ones([4, 128, 512], np.float32)
        maskb = np.empty([4, 128, 256], np.float32)
        for t in range(4):
            mlo = (128 * t + p[:, None] <= qlo[None, :]).astype(np.float32)
            maska[t, :, 0:128] = mlo
            maska[t, :, 256:384] = mlo
        for t in range(4, 8):
            mhi = (128 * t + p[:, None] <= qhi[None, :]).astype(np.float32)
            maskb[t - 4, :, 0:128] = mhi
            maskb[t - 4, :, 128:256] = mhi
        m["maska"] = maska
        m["maskb"] = maskb
        in_maps.append(m)
    return in_maps, with_bias


def _get_program(with_bias, depth_mult=1, fake_coll=False):
    key = ("prog", with_bias, depth_mult, fake_coll)
    if key not in _CACHE:
        _CACHE[key] = _build_program(with_bias, depth_mult, fake_coll)
    return _CACHE[key]


def _assemble(res):
    out = np.empty((B, T, HID), np.float32)
    for c in range(NCORES):
        r, b = c % GSZ, c // GSZ
        o = res[c]["oxt"]  # [HID, TOK]
        out[b, 128 * r:128 * (r + 1), :] = o[:, 0:128].T
        out[b, 128 * (7 - r):128 * (8 - r), :] = o[:, 128:256].T
    return out


def kernel(**inputs):
    from concourse import bass_utils
    in_maps, with_bias = _prepare_inputs(inputs)
    nc = _get_program(with_bias)
    r = bass_utils.run_bass_kernel_spmd(nc, in_maps,
                                        core_ids=list(range(NCORES)))
    return np.ascontiguousarray(_assemble(r.results))
